# revision 10
# baseline (speedup 1.0000x reference)
"""VQ codebook nearest-neighbor (argmin) kernel for 8 Trainium2 NeuronCores.

Math: indices[n] = argmin_k ( ||c_k||^2 - 2 x_n . c_k )
            = argmax_k ( 2 x_n . c_k - ||c_k||^2 )

Per core (data-parallel over N=32768 rows -> 4096 rows/core):
  - PE computes s = x . (2c)^T with a bf16 hi/lo 3-term split
    (x_h c_h + x_h c_l + x_l c_h), fp32 PSUM accumulation -> fp32-class
    accuracy (~3e-8 abs), verified to reproduce the fp64 argmin exactly.
  - DVE adds the broadcast bias (-||c||^2) while copying PSUM->SBUF,
    then finds the row max (top-8) and its first-occurrence index
    via the native max / max_index instructions.
Ties resolve to the lowest index, matching jnp.argmin.
"""
import os
import sys
import numpy as np
import ml_dtypes

sys.path.insert(0, "/opt/trn_rl_repo")
sys.path.insert(0, "/opt/trn_rl_repo/concourse")

import concourse.bass as bass  # noqa: E402
import concourse.mybir as mybir  # noqa: E402
from concourse import bacc  # noqa: E402
from concourse.tile import TileContext  # noqa: E402
from concourse.bass_utils import run_bass_kernel_spmd  # noqa: E402

P = 128          # partitions
D = 512          # feature dim
K = 8192         # codebook entries
N_CORES = 8
NPC = 4096       # rows per core
G = 2048         # psum group width (4 banks)
HALF = 4096      # dists half width

BF16 = ml_dtypes.bfloat16


def build_nc(nt: int, rep: int = 1):
    """Build the Bass module processing `nt` row-tiles of 128 rows.
    rep>1 repeats the whole computation (for benchmarking only)."""
    nc = bacc.Bacc("TRN2", target_bir_lowering=False)
    d_xh = nc.dram_tensor("xh", [D, NPC], mybir.dt.bfloat16, kind="ExternalInput")
    d_xl = nc.dram_tensor("xl", [D, NPC], mybir.dt.bfloat16, kind="ExternalInput")
    d_ch = nc.dram_tensor("ch", [D, K], mybir.dt.bfloat16, kind="ExternalInput")
    d_cl = nc.dram_tensor("cl", [D, K], mybir.dt.bfloat16, kind="ExternalInput")
    d_bias = nc.dram_tensor("bias", [P, K], mybir.dt.float32, kind="ExternalInput")
    d_idx = nc.dram_tensor("idx", [NPC], mybir.dt.int32, kind="ExternalOutput")

    with TileContext(nc) as tc:
        with tc.tile_pool(name="cbp", bufs=1) as cbp, \
             tc.tile_pool(name="xp", bufs=4) as xp, \
             tc.tile_pool(name="dp", bufs=1) as dp, \
             tc.tile_pool(name="sm", bufs=2) as sm, \
             tc.tile_pool(name="outp", bufs=1) as outp, \
             tc.tile_pool(name="pp", bufs=2, space="PSUM") as pp:

            t_ch_c, t_cl_c = [], []
            for c in range(4):
                tch = cbp.tile([P, K], mybir.dt.bfloat16, tag=f"ch{c}",
                               name=f"t_ch_{c}")
                nc.sync.dma_start(tch[:], d_ch[c * P:(c + 1) * P, :])
                t_ch_c.append(tch)
                tcl = cbp.tile([P, K], mybir.dt.bfloat16, tag=f"cl{c}",
                               name=f"t_cl_{c}")
                nc.sync.dma_start(tcl[:], d_cl[c * P:(c + 1) * P, :])
                t_cl_c.append(tcl)
            t_bias = cbp.tile([P, K], mybir.dt.float32, tag="bias")
            nc.sync.dma_start(t_bias[:], d_bias[:])

            out_f = outp.tile([P, nt], mybir.dt.float32, tag="outf")

            for t in [t for _ in range(rep) for t in range(nt)]:
                t_xh = xp.tile([P, 4, P], mybir.dt.bfloat16, tag="xh")
                nc.sync.dma_start(
                    t_xh[:],
                    d_xh[:, t * P:(t + 1) * P].rearrange("(c p) n -> p c n", p=P))
                t_xl = xp.tile([P, 4, P], mybir.dt.bfloat16, tag="xl")
                nc.sync.dma_start(
                    t_xl[:],
                    d_xl[:, t * P:(t + 1) * P].rearrange("(c p) n -> p c n", p=P))

                m8 = sm.tile([P, 2, 8], mybir.dt.float32, tag="m8")
                i8 = sm.tile([P, 2, 8], mybir.dt.uint32, tag="i8")

                for h in range(2):
                    dists = dp.tile([P, HALF], mybir.dt.float32, tag="dists")
                    for gi in range(2):
                        g = h * 2 + gi
                        koff = g * G
                        ps_t = pp.tile([P, G], mybir.dt.float32, tag="ps")
                        # 3-term bf16 split matmuls; snake the weight order
                        # across groups so the boundary LDWEIGHTS dedups.
                        units = [("xh", d) for d in range(4)] + \
                                [("xl", d) for d in range(4)]
                        if g % 2 == 1:
                            units = units[::-1]
                        for ui, (w, d) in enumerate(units):
                            lhsT = t_xh[:, d] if w == "xh" else t_xl[:, d]
                            rhss = ([t_ch_c[d], t_cl_c[d]] if w == "xh"
                                    else [t_ch_c[d]])
                            for ri, rt in enumerate(rhss):
                                for s in range(4):
                                    nc.tensor.matmul(
                                        ps_t[:, s * 512:(s + 1) * 512],
                                        lhsT=lhsT,
                                        rhs=rt[:, koff + s * 512:
                                               koff + (s + 1) * 512],
                                        start=(ui == 0 and ri == 0),
                                        stop=(ui == len(units) - 1
                                              and ri == len(rhss) - 1))
                        # fused bias-add + PSUM->SBUF copy
                        nc.vector.tensor_add(
                            dists[:, gi * G:(gi + 1) * G], ps_t[:],
                            t_bias[:, koff:koff + G])
                    nc.vector.max(out=m8[:, h], in_=dists[:])
                    nc.vector.max_index(i8[:, h], m8[:, h], dists[:])

                # combine halves: k = m0 >= m1 ? i0 : i1 + 4096  (ties -> lower)
                i0f = sm.tile([P, 1], mybir.dt.float32, tag="i0f")
                nc.vector.tensor_copy(i0f[:], i8[:, 0, 0:1])
                i1f = sm.tile([P, 1], mybir.dt.float32, tag="i1f")
                nc.vector.tensor_copy(i1f[:], i8[:, 1, 0:1])
                geq = sm.tile([P, 1], mybir.dt.float32, tag="geq")
                nc.vector.tensor_tensor(
                    geq[:], m8[:, 0, 0:1], m8[:, 1, 0:1], mybir.AluOpType.is_ge)
                nc.vector.tensor_scalar_add(i1f[:], i1f[:], 4096.0)
                dif = sm.tile([P, 1], mybir.dt.float32, tag="dif")
                nc.vector.tensor_sub(dif[:], i0f[:], i1f[:])
                nc.vector.tensor_mul(dif[:], geq[:], dif[:])
                nc.vector.tensor_add(out_f[:, t:t + 1], i1f[:], dif[:])

            out_i = outp.tile([P, nt], mybir.dt.int32, tag="outi")
            nc.vector.tensor_copy(out_i[:], out_f[:])
            nc.sync.dma_start(
                d_idx[0:nt * P].rearrange("(t p) -> p t", p=P), out_i[:])

    _dedup_ldweights(nc)
    nc.compile()
    return nc


def _dedup_ldweights(nc):
    """Drop InstLdweights whose weights AP and deps match the previous LDW in
    PE program order (bf16 matmuls are non-self-loading: the array keeps the
    last loaded weights, so a redundant reload is pure overhead)."""
    n_del = 0
    for f in nc.m.functions:
        stack = [f.blocks]
        while stack:
            blocks = stack.pop()
            for b in blocks:
                new = []
                prev_key = None
                for i in b.instructions:
                    nm = type(i).__name__
                    if nm == "InstLdweights":
                        key = (str(i.ins[0]), tuple(i.sync_dependency_names()))
                        if key == prev_key:
                            n_del += 1
                            continue
                        prev_key = key
                    new.append(i)
                    sub = getattr(i, "blocks", None)
                    if sub:
                        stack.append(sub)
                b.instructions[:] = new
    return n_del


_NC_CACHE = {}


def _get_nc(nt: int):
    rep = int(os.environ.get("VQ_REP", "1")) if os.environ.get("VQ_DEV") else 1
    if (nt, rep) not in _NC_CACHE:
        _NC_CACHE[(nt, rep)] = build_nc(nt, rep)
    return _NC_CACHE[(nt, rep)]


def prep_inputs(x: np.ndarray, codebook: np.ndarray, nt: int = 32):
    """Host-side shard + transpose + bf16 hi/lo split."""
    x = np.asarray(x)
    codebook = np.asarray(codebook)
    flat = np.ascontiguousarray(x.reshape(-1, D).astype(np.float32, copy=False))
    cb = codebook.astype(np.float32, copy=False)

    c2T = np.ascontiguousarray(cb.T) * np.float32(2.0)       # [D, K]
    ch = c2T.astype(BF16)
    cl = (c2T - ch.astype(np.float32)).astype(BF16)
    cb_sqr = np.sum(cb.astype(np.float64) ** 2, axis=1)
    bias = np.ascontiguousarray(
        np.broadcast_to((-cb_sqr).astype(np.float32), (P, K)))

    in_maps = []
    for c in range(N_CORES):
        shard = flat[c * NPC:(c + 1) * NPC]                   # [NPC, D]
        xT = np.ascontiguousarray(shard.T)                    # [D, NPC]
        xh = xT.astype(BF16)
        xl = (xT - xh.astype(np.float32)).astype(BF16)
        in_maps.append({"xh": xh, "xl": xl, "ch": ch, "cl": cl, "bias": bias})
    return in_maps


def kernel(x: np.ndarray, codebook: np.ndarray) -> np.ndarray:
    x = np.asarray(x)
    codebook = np.asarray(codebook)
    nt = int(os.environ.get("VQ_NT", "32")) if os.environ.get("VQ_DEV") else 32
    nc = _get_nc(nt)
    in_maps = prep_inputs(x, codebook, nt)
    trace = bool(int(os.environ.get("VQ_TRACE", "0")))
    res = run_bass_kernel_spmd(
        nc, in_maps, core_ids=list(range(N_CORES)), trace=trace)
    if trace and res.exec_time_ns is not None:
        kernel.last_exec_time_ns = res.exec_time_ns
        kernel.last_results = res
    idx = np.concatenate([r["idx"] for r in res.results])     # [8*NPC]
    if nt == 32:
        return idx.reshape(x.shape[:-1]).astype(np.int32)
    return idx  # partial (dev mode)


kernel.last_exec_time_ns = None
kernel.last_results = None
